# revision 18
# baseline (speedup 1.0000x reference)
"""Trainium2 Bass kernel for CAGKE (Gaussian-kernel spike embedding).

Math: psedu[t] = sum_d softmax(weight)[d] * (spikes (*) K_d)[t] + noise[t],
followed by global min-max normalization. The softmax weights do not depend
on t, so the weighted sum over the D=128 kernel bank commutes with the
convolution: psedu = spikes (*) kbar + noise, where
kbar(delta) = sum_d sw_d * (C/sigma_d) * exp(-(delta-1)^2 / (2 sigma_d^2)).

kbar is truncated to a 45-tap window (|delta| <= 22): the widest Gaussian in
the bank is sigma = 3.0, whose delta = 23 tap is exp(-29.4) ~ 1.7e-13 of the
peak -- far below the 2e-2 tolerance and below f32 resolution of the sum.
The conv is 3 banded 128-contraction matmuls with the partition-flipped
spikes stationary and the kbar Toeplitz bank moving (psedu_rm[c, p] =
sum_qt spf[qt, c+b] * L_k[qt, p]); the Toeplitz bank comes from a DRAM
bounce (overlapping-window reads are only well-defined on the DRAM side of
a DMA, and the BIR verifier only allows ascending outer strides, hence the
flipped contraction).

Latency-focused layout (the whole kernel is one serial dependency chain;
every DMA leg costs 1-2us of issue+completion latency):
 - sigma and weight are fetched as single-descriptor [1,128] rows into two
   partitions of one tile and PE-transposed to [128, 2], instead of a
   128-descriptor per-element spray.
 - the softmax numerator exp(w_d) is applied per-partition: the contraction
   lhsT is v_d = C * exp(w_d) / sigma_d, so the baseline's weight-broadcast
   matmul and accumulator read disappear.  An extra rhs column sigma_d/C
   makes the same matmul emit esum = sum_d exp(w_d), which scales the noise
   (min-max normalization forgives the global esum factor).
 - the Toeplitz readback is descriptor-latency-bound (one descriptor per
   overlapping row), so only live rows are fetched (21 + 64 + 64 + 23) and
   the full-height middle band is split across both HWDGE rings; the
   SWDGE/gpsimd ring (~1.5us slower per DMA) is not used at all.
 - the kernel bank and conv contract in bf16 (1 cycle/row vs 4 for
   double-pumped fp32, half the readback bytes); spikes are 0/1, exact in
   bf16, and the resulting ~1e-3 output error sits far inside the 2e-2
   gate.  Everything around the conv stays f32.
 - the global min/max uses gpsimd's cross-partition reduce (axis=XYZWC)
   instead of a PE transpose round trip, and [-gmin, gmax] are broadcast
   with a single 1-partition matmul.
 - the final normalize + store is split in halves across both rings.

All 8 cores run the identical replicated program (total I/O is ~100KB, far
below the point where sharding would beat collective/sync overhead); the
host takes core 0's output.
"""

import os
import sys

for _p in ("/opt/trn_rl_repo", "/root/.axon_site/_ro/trn_rl_repo"):
    if os.path.isdir(_p) and _p not in sys.path:
        sys.path.insert(0, _p)

import numpy as np

T = 8192  # in_length
D = 128  # embed_dim (kernel bank size)
GAUSS_C = 0.39894228  # 1/sqrt(2*pi) as hardcoded in the source module
NCORES = 8
COLS = T // 128  # 64 columns of 128 contiguous time steps
KW = 511  # kbar buffer, delta in [-255, 255]
J0, JW = 234, 45  # nonzero kbar window: j in [234, 279) -> delta in [-22, 23)
REPL = 8  # kbar replicas spread across DRAM pages (HBM bank parallelism)
RB = 2048  # replica stride in elements (4KB in bf16 = separate DRAM pages)
HC = COLS // 2  # output half for the split store

_CACHE = {}


def _build_bass():
    import concourse.bass as bass
    import concourse.bass_isa as bass_isa
    import concourse.tile as tile
    from concourse import bacc, mybir
    from concourse.bass import _add_dep_helper as add_dep

    f32 = mybir.dt.float32
    bf16 = mybir.dt.bfloat16
    nc = bacc.Bacc("TRN2", target_bir_lowering=False, debug=False, num_devices=NCORES)

    x_d = nc.dram_tensor("X", [1, T], f32, kind="ExternalInput")
    w_d = nc.dram_tensor("weight", [1, D], f32, kind="ExternalInput")
    n_d = nc.dram_tensor("noise", [1, T], f32, kind="ExternalInput")
    s_d = nc.dram_tensor("sigma", [D], f32, kind="ExternalInput")
    o_d = nc.dram_tensor("out", [1, T], f32, kind="ExternalOutput")

    kb_d = nc.dram_tensor("kb_scratch", [RB * (REPL - 1) + KW], bf16)  # replicated DRAM bounce

    with tile.TileContext(nc) as tc:
        with (
            tc.tile_pool(name="sb", bufs=1) as sb,
            tc.tile_pool(name="ps", bufs=1, space="PSUM") as ps,
        ):
            # ---- input DMAs, most-critical first; two HWDGE rings ----
            sw2 = sb.tile([2, 128], f32)  # row 0 = sigma, row 1 = weight
            nc.sync.dma_start(out=sw2[0:1, :], in_=s_d.ap().unsqueeze(0))
            nc.scalar.dma_start(out=sw2[1:2, :], in_=w_d.ap())
            m_x = sb.tile([COLS, 128], f32)
            nc.sync.dma_start(
                out=m_x[:], in_=x_d.ap().rearrange("a (c p) -> (a c) p", p=128)
            )
            nrm = sb.tile([COLS, 128], f32)
            nc.scalar.dma_start(
                out=nrm[:], in_=n_d.ap().rearrange("a (c p) -> (a c) p", p=128)
            )
            # kbar-buffer zeros do not depend on kbar: write them early so
            # the critical kbar write is only the 45-tap window
            zer = sb.tile([REPL, KW - JW], bf16)
            nc.vector.memset(zer[:], 0.0)
            nc.sync.dma_start(
                out=bass.AP(
                    tensor=kb_d.ap().tensor, offset=0, ap=[[RB, REPL], [1, J0]]
                ),
                in_=zer[:, 0:J0],
            )
            nc.scalar.dma_start(
                out=bass.AP(
                    tensor=kb_d.ap().tensor, offset=J0 + JW,
                    ap=[[RB, REPL], [1, KW - J0 - JW]],
                ),
                in_=zer[:, 0 : KW - J0 - JW],
            )
            kbw = sb.tile([REPL, JW], bf16)  # kbar window, bf16

            # ---- gpsimd constants in criticality order: iota feeds dsq;
            # id64's top-left corner is the [2,2] transpose identity ----
            jj = sb.tile([D, JW], f32)  # j - 256, exact in f32
            nc.gpsimd.iota(
                jj[:], pattern=[[1, JW]], base=J0 - 256, channel_multiplier=0,
                allow_small_or_imprecise_dtypes=True,
            )
            id64 = sb.tile([COLS, COLS], f32)
            nc.gpsimd.memset(id64[:], 0.0)
            nc.gpsimd.affine_select(
                out=id64[:], in_=id64[:], compare_op=mybir.AluOpType.not_equal,
                fill=1.0, base=0, pattern=[[-1, COLS]], channel_multiplier=1,
            )
            jx128 = sb.tile([128, 128], f32)  # exchange matrix (anti-diagonal)
            nc.gpsimd.memset(jx128[:], 0.0)
            nc.gpsimd.affine_select(
                out=jx128[:], in_=jx128[:], compare_op=mybir.AluOpType.not_equal,
                fill=1.0, base=-127, pattern=[[1, 128]], channel_multiplier=1,
            )
            ones1 = sb.tile([1, COLS], f32)
            nc.gpsimd.memset(ones1[:], 1.0)
            # Toeplitz bank L[qt, 128k + p] = kbar[qt + p + 128k]; the two
            # corners outside the DMA'd row-bands are dead (zero) regions
            lt = sb.tile([128, 384], bf16, tag="LT")
            nc.gpsimd.memset(lt[0:64, 0:128], 0.0)
            nc.gpsimd.memset(lt[64:128, 256:384], 0.0)
            spf = sb.tile([128, COLS + 2], bf16)  # zero halo columns at 0 and 65
            nc.gpsimd.memset(spf[:, 0:1], 0.0)
            nc.gpsimd.memset(spf[:, COLS + 1 : COLS + 2], 0.0)

            dsq = sb.tile([D, JW], f32)
            nc.vector.tensor_mul(dsq[:], jj[:], jj[:])  # (j - 256)^2

            # ---- sigma/weight onto partitions via one PE transpose ----
            swt = ps.tile([128, 2], f32, tag="ps_a")
            i_swt = nc.tensor.transpose(swt[:], sw2[:], id64[0:2, 0:2])

            # the whole kbar chain runs at elevated scheduler priority: the
            # cost model does not see DMA completion latency, so it otherwise
            # lets spike-path work cut ahead on the in-order engines
            with tc.high_priority():
                inv_sig = sb.tile([D, 1], f32)
                i_recip = nc.vector.reciprocal(inv_sig[:], swt[:, 0:1])
                nhalf = sb.tile([D, 1], f32)  # -1/(2 sigma^2)
                nc.vector.scalar_tensor_tensor(
                    out=nhalf[:], in0=inv_sig[:], scalar=-0.5, in1=inv_sig[:],
                    op0=mybir.AluOpType.mult, op1=mybir.AluOpType.mult,
                )
                expm = sb.tile([D, JW], bf16)
                exp_w = sb.tile([D, 1], f32)  # softmax numerator e_d
                i_expw = nc.scalar.activation(
                    out=exp_w[:], in_=swt[:, 1:2],
                    func=mybir.ActivationFunctionType.Exp, bias=0.0, scale=1.0,
                )
                i_expm = nc.scalar.activation(
                    out=expm[:], in_=dsq[:],
                    func=mybir.ActivationFunctionType.Exp,
                    bias=0.0, scale=nhalf[:, 0:1],
                )  # per-sigma gaussian row
                v8 = sb.tile([D, REPL], bf16)  # C * e_d / sigma_d, x8 cols
                _expw_b = exp_w[:]
                _inv_b = inv_sig[:]
                i_v8 = nc.vector.scalar_tensor_tensor(
                    out=v8[:],
                    in0=bass.AP(
                        tensor=_expw_b.tensor, offset=_expw_b.offset,
                        ap=[[1, D], [0, REPL]],
                    ),
                    scalar=GAUSS_C,
                    in1=bass.AP(
                        tensor=_inv_b.tensor, offset=_inv_b.offset,
                        ap=[[1, D], [0, REPL]],
                    ),
                    op0=mybir.AluOpType.mult, op1=mybir.AluOpType.mult,
                )

                sigc = sb.tile([D, 1], bf16)  # sigma/C: esum contraction column
                nc.vector.tensor_scalar(
                    out=sigc[:], in0=swt[:, 0:1],
                    scalar1=1.0 / GAUSS_C, scalar2=None,
                    op0=mybir.AluOpType.mult,
                )

                # ---- kbar row (bf16 matmul, REPL output rows so the DRAM
                # write lands in 8 page-spread replicas); esum = v . sigma/C
                # via a second tiny matmul off the critical path ----
                kb_ps = ps.tile([REPL, JW], f32, tag="ps_b")
                i_kbmm = nc.tensor.matmul(
                    kb_ps[:], lhsT=v8[:], rhs=expm[:], start=True, stop=True,
                )
                i_kbcopy = nc.vector.tensor_copy(kbw[:], kb_ps[:])
                nc.sync.dma_start(
                    out=bass.AP(
                        tensor=kb_d.ap().tensor, offset=J0,
                        ap=[[RB, REPL], [1, JW]],
                    ),
                    in_=kbw[:],
                )
                es_ps1 = ps.tile([1, 1], f32, tag="ps_g")
                nc.tensor.matmul(
                    es_ps1[:], lhsT=v8[:, 0:1], rhs=sigc[:], start=True, stop=True,
                )
                esum_sb = sb.tile([1, 1], f32)
                nc.vector.tensor_copy(esum_sb[:], es_ps1[:])
                # L[qt, 128k + p] = kbar[qt + p + 128k] (flipped contraction
                # qt = 127-q; the spike operand is partition-flipped to
                # match). Each fetched row is one contiguous 512B window of
                # kb_d, so the whole bank is two 64-descriptor DMAs, one per
                # HWDGE ring; the skipped corners are pure zeros (memset).
                nc.sync.dma_start(
                    out=lt[0:64, 128:384],
                    in_=bass.AP(
                        tensor=kb_d.ap().tensor, offset=128,
                        ap=[[8, 8], [RB + 1, REPL], [1, 256]],
                    ),
                )
                nc.scalar.dma_start(
                    out=lt[64:128, 0:256],
                    in_=bass.AP(
                        tensor=kb_d.ap().tensor, offset=64,
                        ap=[[8, 8], [RB + 1, REPL], [1, 256]],
                    ),
                )

            # ---- spikes: threshold, PE transpose, partition flip ----
            spk = sb.tile([COLS, 128], f32)
            i_thr = nc.vector.tensor_scalar(
                out=spk[:], in0=m_x[:], scalar1=0.5, scalar2=None,
                op0=mybir.AluOpType.is_gt,
            )
            add_dep(i_thr.ins, i_v8.ins, sync=False, reason="kbar chain first")
            sp_ps = ps.tile([128, COLS], f32, tag="ps_c")
            i_spt = nc.tensor.transpose(sp_ps[:], spk[:], id64[:])
            sp_sb = sb.tile([128, COLS], f32)
            i_spcopy = nc.vector.tensor_copy(sp_sb[:], sp_ps[:])
            spf_ps = ps.tile([128, COLS], f32, tag="ps_d")
            i_spf = nc.tensor.matmul(
                spf_ps[:], lhsT=jx128[:], rhs=sp_sb[:], start=True, stop=True,
            )  # partition-flip: spf_ps[qt, c] = spikes[128c + 127 - qt]
            i_spfcopy = nc.vector.tensor_copy(spf[:, 1 : COLS + 1], spf_ps[:])
            # keep the in-order DVE and PE streams from letting spike-path
            # work delay the serial kbar chain
            add_dep(i_thr.ins, i_recip.ins, sync=False, reason="kbar chain first")
            add_dep(i_spcopy.ins, i_kbcopy.ins, sync=False, reason="kbar chain first")
            add_dep(i_spfcopy.ins, i_kbcopy.ins, sync=False, reason="kbar chain first")
            add_dep(i_spt.ins, i_kbmm.ins, sync=False, reason="kbar chain first")

            # ---- esum broadcast onto the 64 output partitions ----
            es_ps = ps.tile([COLS, 1], f32, tag="ps_e")
            i_esmm = nc.tensor.matmul(
                es_ps[:], lhsT=ones1[:], rhs=esum_sb[:], start=True, stop=True,
            )
            add_dep(i_esmm.ins, i_kbmm.ins, sync=False, reason="kbar chain first")

            # ---- banded conv, row-major output: spikes stationary, kbar
            # moving: psedu_rm[c, p] = sum_qt spf[qt, c+b] * L_k[qt, p] ----
            conv_ps = ps.tile([COLS, 128], f32, tag="ps_f")
            for i, (k, b) in enumerate(((0, 1), (1, 0), (2, -1))):
                nc.tensor.matmul(
                    conv_ps[:],
                    lhsT=spf[:, 1 + b : COLS + 1 + b],
                    rhs=lt[:, 128 * k : 128 * (k + 1)],
                    start=(i == 0),
                    stop=(i == 2),
                )

            # ---- add esum-scaled noise (still row-major) ----
            ps_rm = sb.tile([COLS, 128], f32)
            nc.vector.scalar_tensor_tensor(
                out=ps_rm[:], in0=nrm[:], scalar=es_ps[:, 0:1], in1=conv_ps[:],
                op0=mybir.AluOpType.mult, op1=mybir.AluOpType.add,
            )  # esum * (conv_true + noise_true) up to the global scale

            # ---- global min/max: per-partition [max, -min] on DVE, then
            # one gpsimd cross-partition max (C-axis reduce supports only
            # add/average/max, hence the negated min) ----
            pk = sb.tile([COLS, 2], f32)
            nc.vector.tensor_reduce(
                out=pk[:, 0:1], in_=ps_rm[:], axis=mybir.AxisListType.X,
                op=mybir.AluOpType.max,
            )
            nc.vector.tensor_reduce(
                out=pk[:, 1:2], in_=ps_rm[:], axis=mybir.AxisListType.X,
                op=mybir.AluOpType.min, negate=True,
            )
            stat_sb = sb.tile([COLS, 2], f32)  # [gmax, -gmin] on every partition
            nc.gpsimd.partition_all_reduce(
                stat_sb[:], pk[:], channels=COLS, reduce_op=bass_isa.ReduceOp.max,
            )
            rng = sb.tile([COLS, 1], f32)
            nc.vector.tensor_scalar(
                out=rng[:], in0=stat_sb[:, 0:1], scalar1=stat_sb[:, 1:2],
                scalar2=None, op0=mybir.AluOpType.add,
            )
            inv_rng = sb.tile([COLS, 1], f32)
            nc.vector.reciprocal(inv_rng[:], rng[:])
            # out = (psedu + (-gmin)) * (1/range), stored in halves on both
            # rings so the second half's DMA issue overlaps the first's
            outt = sb.tile([COLS, 128], f32)
            nc.vector.tensor_scalar(
                out=outt[0:HC, :], in0=ps_rm[0:HC, :], scalar1=stat_sb[0:HC, 1:2],
                scalar2=inv_rng[0:HC, 0:1], op0=mybir.AluOpType.add,
                op1=mybir.AluOpType.mult,
            )
            nc.sync.dma_start(
                out=o_d.ap()[:, 0 : T // 2].rearrange("a (c p) -> (a c) p", p=128),
                in_=outt[0:HC, :],
            )
            nc.gpsimd.tensor_scalar(
                out=outt[HC:COLS, :], in0=ps_rm[HC:COLS, :],
                scalar1=stat_sb[HC:COLS, 1:2], scalar2=inv_rng[HC:COLS, 0:1],
                op0=mybir.AluOpType.add, op1=mybir.AluOpType.mult,
            )
            nc.scalar.dma_start(
                out=o_d.ap()[:, T // 2 : T].rearrange("a (c p) -> (a c) p", p=128),
                in_=outt[HC:COLS, :],
            )

    nc.compile()
    return nc


def _get_nc():
    if "nc" not in _CACHE:
        _CACHE["nc"] = _build_bass()
    return _CACHE["nc"]


def _run(in_map, trace=False, **kwargs):
    from concourse.bass_utils import run_bass_kernel_spmd

    nc = _get_nc()
    return run_bass_kernel_spmd(
        nc, [in_map] * NCORES, core_ids=list(range(NCORES)), trace=trace, **kwargs
    )


def kernel(X, weight, noise, sigma):
    in_map = {
        "X": np.ascontiguousarray(X, dtype=np.float32).reshape(1, T),
        "weight": np.ascontiguousarray(weight, dtype=np.float32).reshape(1, D),
        "noise": np.ascontiguousarray(noise, dtype=np.float32).reshape(1, T),
        "sigma": np.ascontiguousarray(sigma, dtype=np.float32).reshape(D),
    }
    res = _run(in_map).results
    return res[0]["out"].reshape(1, T)


# revision 19
# speedup vs baseline: 1.1680x; 1.1680x over previous
"""Trainium2 Bass kernel for CAGKE (Gaussian-kernel spike embedding).

Math: psedu[t] = sum_d softmax(weight)[d] * (spikes (*) K_d)[t] + noise[t],
followed by global min-max normalization. The softmax weights do not depend
on t, so the weighted sum over the D=128 kernel bank commutes with the
convolution: psedu = spikes (*) kbar + noise, where
kbar(delta) = sum_d sw_d * (C/sigma_d) * exp(-(delta-1)^2 / (2 sigma_d^2)).

kbar is truncated to a 45-tap window (|delta| <= 22): the widest Gaussian in
the bank is sigma = 3.0, whose delta = 23 tap is exp(-29.4) ~ 1.7e-13 of the
peak -- far below the 2e-2 tolerance and below f32 resolution of the sum.
The conv is 3 banded 128-contraction matmuls with the partition-flipped
spikes stationary and the kbar Toeplitz bank moving (psedu_rm[c, p] =
sum_qt spf[qt, c+b] * L_k[qt, p]); the Toeplitz bank comes from a DRAM
bounce (overlapping-window reads are only well-defined on the DRAM side of
a DMA, and the BIR verifier only allows ascending outer strides, hence the
flipped contraction).

Latency-focused layout (the whole kernel is one serial dependency chain;
every DMA leg costs 1-2us of issue+completion latency):
 - sigma and weight are fetched as single-descriptor [1,128] rows into two
   partitions of one tile and PE-transposed to [128, 2], instead of a
   128-descriptor per-element spray.
 - the softmax numerator exp(w_d) is applied per-partition: the contraction
   lhsT is v_d = C * exp(w_d) / sigma_d, so the baseline's weight-broadcast
   matmul and accumulator read disappear.  An extra rhs column sigma_d/C
   makes the same matmul emit esum = sum_d exp(w_d), which scales the noise
   (min-max normalization forgives the global esum factor).
 - the Toeplitz readback is descriptor-latency-bound (one descriptor per
   overlapping row), so only live rows are fetched (21 + 64 + 64 + 23) and
   the full-height middle band is split across both HWDGE rings; the
   SWDGE/gpsimd ring (~1.5us slower per DMA) is not used at all.
 - the kernel bank and conv contract in bf16 (1 cycle/row vs 4 for
   double-pumped fp32, half the readback bytes); spikes are 0/1, exact in
   bf16, and the resulting ~1e-3 output error sits far inside the 2e-2
   gate.  Everything around the conv stays f32.
 - the global min/max uses gpsimd's cross-partition reduce (axis=XYZWC)
   instead of a PE transpose round trip, and [-gmin, gmax] are broadcast
   with a single 1-partition matmul.
 - the final normalize + store is split in halves across both rings.

All 8 cores run the identical replicated program (total I/O is ~100KB, far
below the point where sharding would beat collective/sync overhead); the
host takes core 0's output.
"""

import os
import sys

for _p in ("/opt/trn_rl_repo", "/root/.axon_site/_ro/trn_rl_repo"):
    if os.path.isdir(_p) and _p not in sys.path:
        sys.path.insert(0, _p)

import numpy as np

T = 8192  # in_length
D = 128  # embed_dim (kernel bank size)
GAUSS_C = 0.39894228  # 1/sqrt(2*pi) as hardcoded in the source module
NCORES = 8
COLS = T // 128  # 64 columns of 128 contiguous time steps
KW = 511  # kbar buffer, delta in [-255, 255]
J0, JW = 234, 45  # nonzero kbar window: j in [234, 279) -> delta in [-22, 23)
REPL = 8  # kbar replicas spread across DRAM pages (HBM bank parallelism)
RB = 2048  # replica stride in elements (4KB in bf16 = separate DRAM pages)
HC = COLS // 2  # output half for the split store

_CACHE = {}


def _build_bass():
    import concourse.bass as bass
    import concourse.bass_isa as bass_isa
    import concourse.tile as tile
    from concourse import bacc, mybir
    from concourse.bass import _add_dep_helper as add_dep

    f32 = mybir.dt.float32
    bf16 = mybir.dt.bfloat16
    nc = bacc.Bacc("TRN2", target_bir_lowering=False, debug=False, num_devices=NCORES)

    x_d = nc.dram_tensor("X", [1, T], f32, kind="ExternalInput")
    w_d = nc.dram_tensor("weight", [1, D], f32, kind="ExternalInput")
    n_d = nc.dram_tensor("noise", [1, T], f32, kind="ExternalInput")
    s_d = nc.dram_tensor("sigma", [D], f32, kind="ExternalInput")
    o_d = nc.dram_tensor("out", [1, T], f32, kind="ExternalOutput")

    kb_d = nc.dram_tensor("kb_scratch", [RB * (REPL - 1) + KW], bf16)  # replicated DRAM bounce

    with tile.TileContext(nc) as tc:
        with (
            tc.tile_pool(name="sb", bufs=1) as sb,
            tc.tile_pool(name="ps", bufs=1, space="PSUM") as ps,
        ):
            # ---- input DMAs, most-critical first; two HWDGE rings ----
            sw2 = sb.tile([2, 128], f32)  # row 0 = sigma, row 1 = weight
            nc.sync.dma_start(out=sw2[0:1, :], in_=s_d.ap().unsqueeze(0))
            nc.scalar.dma_start(out=sw2[1:2, :], in_=w_d.ap())
            m_x = sb.tile([COLS, 128], f32)
            nc.sync.dma_start(
                out=m_x[:], in_=x_d.ap().rearrange("a (c p) -> (a c) p", p=128)
            )
            nrm = sb.tile([COLS, 128], f32)
            nc.scalar.dma_start(
                out=nrm[:], in_=n_d.ap().rearrange("a (c p) -> (a c) p", p=128)
            )
            kbz = sb.tile([REPL, KW], bf16)  # full kbar row: zeros + window
            nc.vector.memset(kbz[:], 0.0)

            # ---- gpsimd constants in criticality order: iota feeds dsq;
            # id64's top-left corner is the [2,2] transpose identity ----
            jj = sb.tile([D, JW], f32)  # j - 256, exact in f32
            nc.gpsimd.iota(
                jj[:], pattern=[[1, JW]], base=J0 - 256, channel_multiplier=0,
                allow_small_or_imprecise_dtypes=True,
            )
            id64 = sb.tile([COLS, COLS], f32)
            nc.gpsimd.memset(id64[:], 0.0)
            nc.gpsimd.affine_select(
                out=id64[:], in_=id64[:], compare_op=mybir.AluOpType.not_equal,
                fill=1.0, base=0, pattern=[[-1, COLS]], channel_multiplier=1,
            )
            jx128 = sb.tile([128, 128], f32)  # exchange matrix (anti-diagonal)
            nc.gpsimd.memset(jx128[:], 0.0)
            nc.gpsimd.affine_select(
                out=jx128[:], in_=jx128[:], compare_op=mybir.AluOpType.not_equal,
                fill=1.0, base=-127, pattern=[[1, 128]], channel_multiplier=1,
            )
            ones1 = sb.tile([1, COLS], f32)
            nc.gpsimd.memset(ones1[:], 1.0)
            # Toeplitz bank L[qt, 128k + p] = kbar[qt + p + 128k]; the two
            # corners outside the DMA'd row-bands are dead (zero) regions
            lt = sb.tile([128, 384], bf16, tag="LT")
            nc.gpsimd.memset(lt[0:64, 0:128], 0.0)
            nc.gpsimd.memset(lt[64:128, 256:384], 0.0)
            spf = sb.tile([128, COLS + 2], bf16)  # zero halo columns at 0 and 65
            nc.gpsimd.memset(spf[:, 0:1], 0.0)
            nc.gpsimd.memset(spf[:, COLS + 1 : COLS + 2], 0.0)

            dsq = sb.tile([D, JW], f32)
            nc.vector.tensor_mul(dsq[:], jj[:], jj[:])  # (j - 256)^2

            # ---- sigma/weight onto partitions via one PE transpose ----
            swt = ps.tile([128, 2], f32, tag="ps_a")
            i_swt = nc.tensor.transpose(swt[:], sw2[:], id64[0:2, 0:2])

            # the whole kbar chain runs at elevated scheduler priority: the
            # cost model does not see DMA completion latency, so it otherwise
            # lets spike-path work cut ahead on the in-order engines
            with tc.high_priority():
                inv_sig = sb.tile([D, 1], f32)
                i_recip = nc.vector.reciprocal(inv_sig[:], swt[:, 0:1])
                nhalf = sb.tile([D, 1], f32)  # -1/(2 sigma^2)
                nc.vector.scalar_tensor_tensor(
                    out=nhalf[:], in0=inv_sig[:], scalar=-0.5, in1=inv_sig[:],
                    op0=mybir.AluOpType.mult, op1=mybir.AluOpType.mult,
                )
                expm = sb.tile([D, JW], bf16)
                exp_w = sb.tile([D, 1], f32)  # softmax numerator e_d
                i_expw = nc.scalar.activation(
                    out=exp_w[:], in_=swt[:, 1:2],
                    func=mybir.ActivationFunctionType.Exp, bias=0.0, scale=1.0,
                )
                i_expm = nc.scalar.activation(
                    out=expm[:], in_=dsq[:],
                    func=mybir.ActivationFunctionType.Exp,
                    bias=0.0, scale=nhalf[:, 0:1],
                )  # per-sigma gaussian row
                v8 = sb.tile([D, REPL], bf16)  # C * e_d / sigma_d, x8 cols
                _expw_b = exp_w[:]
                _inv_b = inv_sig[:]
                i_v8 = nc.vector.scalar_tensor_tensor(
                    out=v8[:],
                    in0=bass.AP(
                        tensor=_expw_b.tensor, offset=_expw_b.offset,
                        ap=[[1, D], [0, REPL]],
                    ),
                    scalar=GAUSS_C,
                    in1=bass.AP(
                        tensor=_inv_b.tensor, offset=_inv_b.offset,
                        ap=[[1, D], [0, REPL]],
                    ),
                    op0=mybir.AluOpType.mult, op1=mybir.AluOpType.mult,
                )

                sigc = sb.tile([D, 1], bf16)  # sigma/C: esum contraction column
                nc.vector.tensor_scalar(
                    out=sigc[:], in0=swt[:, 0:1],
                    scalar1=1.0 / GAUSS_C, scalar2=None,
                    op0=mybir.AluOpType.mult,
                )

                # ---- kbar row (bf16 matmul, REPL output rows so the DRAM
                # write lands in 8 page-spread replicas); esum = v . sigma/C
                # via a second tiny matmul off the critical path ----
                kb_ps = ps.tile([REPL, JW], f32, tag="ps_b")
                i_kbmm = nc.tensor.matmul(
                    kb_ps[:], lhsT=v8[:], rhs=expm[:], start=True, stop=True,
                )
                i_kbcopy = nc.vector.tensor_copy(kbz[:, J0 : J0 + JW], kb_ps[:])
                nc.sync.dma_start(
                    out=bass.AP(
                        tensor=kb_d.ap().tensor, offset=0,
                        ap=[[RB, REPL], [1, KW]],
                    ),
                    in_=kbz[:],
                )
                es_ps1 = ps.tile([1, 1], f32, tag="ps_g")
                nc.tensor.matmul(
                    es_ps1[:], lhsT=v8[:, 0:1], rhs=sigc[:], start=True, stop=True,
                )
                esum_sb = sb.tile([1, 1], f32)
                nc.vector.tensor_copy(esum_sb[:], es_ps1[:])
                # L[qt, 128k + p] = kbar[qt + p + 128k] (flipped contraction
                # qt = 127-q; the spike operand is partition-flipped to
                # match). Each fetched row is one contiguous 512B window of
                # kb_d, so the whole bank is two 64-descriptor DMAs, one per
                # HWDGE ring; the skipped corners are pure zeros (memset).
                nc.sync.dma_start(
                    out=lt[0:64, 128:384],
                    in_=bass.AP(
                        tensor=kb_d.ap().tensor, offset=128,
                        ap=[[8, 8], [RB + 1, REPL], [1, 256]],
                    ),
                )
                nc.scalar.dma_start(
                    out=lt[64:128, 0:256],
                    in_=bass.AP(
                        tensor=kb_d.ap().tensor, offset=64,
                        ap=[[8, 8], [RB + 1, REPL], [1, 256]],
                    ),
                )

            # ---- spikes: threshold, PE transpose, partition flip ----
            spk = sb.tile([COLS, 128], f32)
            i_thr = nc.vector.tensor_scalar(
                out=spk[:], in0=m_x[:], scalar1=0.5, scalar2=None,
                op0=mybir.AluOpType.is_gt,
            )
            add_dep(i_thr.ins, i_v8.ins, sync=False, reason="kbar chain first")
            sp_ps = ps.tile([128, COLS], f32, tag="ps_c")
            i_spt = nc.tensor.transpose(sp_ps[:], spk[:], id64[:])
            sp_sb = sb.tile([128, COLS], f32)
            i_spcopy = nc.vector.tensor_copy(sp_sb[:], sp_ps[:])
            spf_ps = ps.tile([128, COLS], f32, tag="ps_d")
            i_spf = nc.tensor.matmul(
                spf_ps[:], lhsT=jx128[:], rhs=sp_sb[:], start=True, stop=True,
            )  # partition-flip: spf_ps[qt, c] = spikes[128c + 127 - qt]
            i_spfcopy = nc.vector.tensor_copy(spf[:, 1 : COLS + 1], spf_ps[:])
            # keep the in-order DVE and PE streams from letting spike-path
            # work delay the serial kbar chain
            add_dep(i_thr.ins, i_recip.ins, sync=False, reason="kbar chain first")
            add_dep(i_spcopy.ins, i_kbcopy.ins, sync=False, reason="kbar chain first")
            add_dep(i_spfcopy.ins, i_kbcopy.ins, sync=False, reason="kbar chain first")
            add_dep(i_spt.ins, i_kbmm.ins, sync=False, reason="kbar chain first")

            # ---- esum broadcast onto the 64 output partitions ----
            es_ps = ps.tile([COLS, 1], f32, tag="ps_e")
            i_esmm = nc.tensor.matmul(
                es_ps[:], lhsT=ones1[:], rhs=esum_sb[:], start=True, stop=True,
            )
            add_dep(i_esmm.ins, i_kbmm.ins, sync=False, reason="kbar chain first")

            # ---- banded conv, row-major output: spikes stationary, kbar
            # moving: psedu_rm[c, p] = sum_qt spf[qt, c+b] * L_k[qt, p] ----
            conv_ps = ps.tile([COLS, 128], f32, tag="ps_f")
            for i, (k, b) in enumerate(((0, 1), (1, 0), (2, -1))):
                nc.tensor.matmul(
                    conv_ps[:],
                    lhsT=spf[:, 1 + b : COLS + 1 + b],
                    rhs=lt[:, 128 * k : 128 * (k + 1)],
                    start=(i == 0),
                    stop=(i == 2),
                )

            # ---- add esum-scaled noise (still row-major) ----
            ps_rm = sb.tile([COLS, 128], f32)
            nc.vector.scalar_tensor_tensor(
                out=ps_rm[:], in0=nrm[:], scalar=es_ps[:, 0:1], in1=conv_ps[:],
                op0=mybir.AluOpType.mult, op1=mybir.AluOpType.add,
            )  # esum * (conv_true + noise_true) up to the global scale

            # ---- global min/max: per-partition [max, -min] on DVE, then
            # one gpsimd cross-partition max (C-axis reduce supports only
            # add/average/max, hence the negated min) ----
            pk = sb.tile([COLS, 2], f32)
            nc.vector.tensor_reduce(
                out=pk[:, 0:1], in_=ps_rm[:], axis=mybir.AxisListType.X,
                op=mybir.AluOpType.max,
            )
            nc.vector.tensor_reduce(
                out=pk[:, 1:2], in_=ps_rm[:], axis=mybir.AxisListType.X,
                op=mybir.AluOpType.min, negate=True,
            )
            stat_sb = sb.tile([COLS, 2], f32)  # [gmax, -gmin] on every partition
            nc.gpsimd.partition_all_reduce(
                stat_sb[:], pk[:], channels=COLS, reduce_op=bass_isa.ReduceOp.max,
            )
            rng = sb.tile([COLS, 1], f32)
            nc.vector.tensor_scalar(
                out=rng[:], in0=stat_sb[:, 0:1], scalar1=stat_sb[:, 1:2],
                scalar2=None, op0=mybir.AluOpType.add,
            )
            inv_rng = sb.tile([COLS, 1], f32)
            nc.vector.reciprocal(inv_rng[:], rng[:])
            # out = (psedu + (-gmin)) * (1/range), stored in halves on both
            # rings so the second half's DMA issue overlaps the first's
            outt = sb.tile([COLS, 128], f32)
            nc.vector.tensor_scalar(
                out=outt[0:HC, :], in0=ps_rm[0:HC, :], scalar1=stat_sb[0:HC, 1:2],
                scalar2=inv_rng[0:HC, 0:1], op0=mybir.AluOpType.add,
                op1=mybir.AluOpType.mult,
            )
            nc.sync.dma_start(
                out=o_d.ap()[:, 0 : T // 2].rearrange("a (c p) -> (a c) p", p=128),
                in_=outt[0:HC, :],
            )
            nc.gpsimd.tensor_scalar(
                out=outt[HC:COLS, :], in0=ps_rm[HC:COLS, :],
                scalar1=stat_sb[HC:COLS, 1:2], scalar2=inv_rng[HC:COLS, 0:1],
                op0=mybir.AluOpType.add, op1=mybir.AluOpType.mult,
            )
            nc.scalar.dma_start(
                out=o_d.ap()[:, T // 2 : T].rearrange("a (c p) -> (a c) p", p=128),
                in_=outt[HC:COLS, :],
            )

    nc.compile()
    # the SWDGE/gpsimd DMA ring is never used; dropping its queue
    # declaration lets the NEFF wrapper skip its ring setup/teardown
    nc.m.queues = [q for q in nc.m.queues if q.name != "qPoolDynamic"]
    return nc


def _get_nc():
    if "nc" not in _CACHE:
        _CACHE["nc"] = _build_bass()
    return _CACHE["nc"]


def _run(in_map, trace=False, **kwargs):
    from concourse.bass_utils import run_bass_kernel_spmd

    nc = _get_nc()
    return run_bass_kernel_spmd(
        nc, [in_map] * NCORES, core_ids=list(range(NCORES)), trace=trace, **kwargs
    )


def kernel(X, weight, noise, sigma):
    in_map = {
        "X": np.ascontiguousarray(X, dtype=np.float32).reshape(1, T),
        "weight": np.ascontiguousarray(weight, dtype=np.float32).reshape(1, D),
        "noise": np.ascontiguousarray(noise, dtype=np.float32).reshape(1, T),
        "sigma": np.ascontiguousarray(sigma, dtype=np.float32).reshape(D),
    }
    res = _run(in_map).results
    return res[0]["out"].reshape(1, T)
